# revision 8
# baseline (speedup 1.0000x reference)
"""Trainium2 Bass kernel for nn_ExtractRelevantPatchesLayer.

Per-image: 64x64 avg-pool on a [1024,1024] heatmap -> top-32 of the 256
pooled values -> gather the corresponding 64x64x3 image patches.

Sharding: batch dim (16) data-parallel across 8 NeuronCores, 2 images per
core, no cross-core communication.

Bit-exactness strategy (pooled values have 1-ULP gaps, so patch selection
must reproduce jax's f32 summation order exactly):
  - jax mean = sequential f32 sum over the 64 contiguous columns, then
    sequential f32 sum over the 64 rows (verified bitwise vs jax CPU+TRN).
  - DVE tensor_reduce is sequential over the free axis (HW-verified).
  - PE transpose (identity matmul) moves f32 bitwise (HW-verified), letting
    a second DVE reduce do the row sums sequentially.
  - top-32 via 4 rounds of max8/max_index/match_replace reproduces
    jax.lax.top_k ordering including duplicate handling (HW-verified).
Only the selected patches are read from HBM (dma_gather of 192-float rows),
so HBM traffic/core is ~8 MiB heatmap + 3 MiB gather + 3 MiB store.
"""
import os
import sys

for p in ("/opt/trn_rl_repo", "/root/.axon_site/_ro/trn_rl_repo"):
    if os.path.isdir(p) and p not in sys.path:
        sys.path.append(p)

import numpy as np

import concourse.bacc as bacc
import concourse.bass_isa as bass_isa
import concourse.mybir as mybir
import concourse.tile as tile
from concourse.bass_utils import run_bass_kernel_spmd

F32 = mybir.dt.float32
I32 = mybir.dt.int32
I16 = mybir.dt.int16
U32 = mybir.dt.uint32

B_LOCAL = 2          # batches per core
N_CORES = 8
P = 64               # patch size
K = 32               # patches kept per batch
GRID = 16            # 16x16 candidate patches
NEG_FILL = -1.0e30

_CACHE: dict = {}


def _build_module():
    nc = bacc.Bacc("TRN2", target_bir_lowering=False, debug=False)

    # Local shard tensors (per core): 2 batches.
    hm_d = nc.dram_tensor("hm", [B_LOCAL * 1024, 1024], F32, kind="ExternalInput")
    img_d = nc.dram_tensor("img", [B_LOCAL * 16384, 192], F32, kind="ExternalInput")
    sel_d = nc.dram_tensor("sel", [B_LOCAL * K, P * P * 3], F32, kind="ExternalOutput")

    # Inline constants.
    ident_np = np.eye(128, dtype=np.float32)
    ident_d = nc.inline_tensor(ident_np, name="ident")
    # iotaA[p, s] = (p%16)*16 + (s%4)*256   (s = j*4 + rr_hi)
    pp, ss = np.meshgrid(np.arange(128), np.arange(128), indexing="ij")
    iota_np = ((pp % 16) * 16 + (ss % 4) * 256).astype(np.int32)
    iota_d = nc.inline_tensor(iota_np, name="iotaA")

    with tile.TileContext(nc) as tc:
        with tc.tile_pool(name="consts", bufs=1) as cpool, \
             tc.tile_pool(name="heat", bufs=4) as hpool, \
             tc.tile_pool(name="work", bufs=1) as wpool, \
             tc.tile_pool(name="gath", bufs=1) as gpool, \
             tc.tile_pool(name="dr", bufs=1, space="DRAM") as dpool, \
             tc.tile_pool(name="ps", bufs=2, space="PSUM") as ppool:

            ident = cpool.tile([128, 128], F32, tag="ident", name="ident")
            nc.sync.dma_start(ident[:], ident_d[:])
            iota = cpool.tile([128, 128], I32, tag="iota", name="iota")
            nc.sync.dma_start(iota[:], iota_d[:])

            # Column partials: P_all[b][p, t*16+gw] = seq-sum over the 64
            # cols of group gw, row (t*128+p) of batch b.
            p_all = [wpool.tile([128, 128], F32, tag=f"pall{b}", name=f"pall{b}") for b in range(B_LOCAL)]
            for t in range(16):
                ht = hpool.tile([128, 1024], F32, tag="heat", name="heat")
                nc.sync.dma_start(ht[:], hm_d[t * 128:(t + 1) * 128, :])
                nc.vector.tensor_reduce(
                    out=p_all[t // 8][:, (t % 8) * 16:((t % 8) + 1) * 16],
                    in_=ht[:].rearrange("p (g c) -> p g c", c=64),
                    axis=mybir.AxisListType.X,
                    op=mybir.AluOpType.add,
                )

            for b in range(B_LOCAL):
                # Row sums: transpose partials so each partition holds one
                # (t, gw) column of 128 row-partials, then reduce per 64.
                pt = ppool.tile([128, 128], F32, tag="pt", name="pt")
                nc.tensor.transpose(pt[:], p_all[b][:], ident[:])
                # Padded to 32 free elems so the second PE transpose is legal.
                sums = wpool.tile([128, 32], F32, tag=f"sums{b}", name=f"sums{b}")
                nc.vector.memset(sums[:], 0.0)
                nc.vector.tensor_reduce(
                    out=sums[:, 0:2],
                    in_=pt[:].rearrange("q (m r) -> q m r", r=64),
                    axis=mybir.AxisListType.X,
                    op=mybir.AluOpType.add,
                )
                # Flatten to [1, 256] in n = gh*16+gw = 32t+16m+gw order.
                # Transpose sums -> [m, q] rows, then interleave on the DRAM
                # side (SBUF APs treat dim0 as partitions, so the interleaved
                # AP is only legal on DRAM).
                pt2 = ppool.tile([32, 128], F32, tag="pt2", name="pt2")
                nc.tensor.transpose(pt2[:], sums[:], ident[:])
                s2 = wpool.tile([2, 128], F32, tag=f"s2{b}", name=f"s2{b}")
                nc.vector.tensor_copy(s2[:], pt2[0:2, :])
                sc = dpool.tile([1, 256], F32, tag=f"sc{b}", name=f"sc{b}")
                nc.sync.dma_start(
                    sc[:].rearrange("o (t m g) -> o m t g", t=8, m=2, g=16),
                    s2[:])
                vflat = wpool.tile([1, 256], F32, tag=f"vflat{b}", name=f"vflat{b}")
                nc.sync.dma_start(vflat[:], sc[:])
                # Replicate to all 128 partitions.
                vrep = wpool.tile([128, 256], F32, tag=f"vrep{b}", name=f"vrep{b}")
                nc.gpsimd.partition_broadcast(vrep[:], vflat[:], channels=128)

                # Top-32, descending, jax tie order.
                idxs = wpool.tile([128, 32], U32, tag=f"idx{b}", name=f"idx{b}")
                for rnd in range(4):
                    mx = wpool.tile([128, 8], F32, tag=f"mx{b}", name=f"mx{b}")
                    nc.vector.max(out=mx[:], in_=vrep[:])
                    nc.vector.max_index(
                        out=idxs[:, rnd * 8:(rnd + 1) * 8],
                        in_max=mx[:], in_values=vrep[:])
                    nc.vector.match_replace(
                        out=vrep[:], in_to_replace=mx[:], in_values=vrep[:],
                        imm_value=NEG_FILL)

                # Gather row indices: k = rowbase + rr_hi*256 + q*16, with
                # rowbase = n + 1008*(n>>4)  (n = gh*16+gw).
                idx_i = wpool.tile([128, 32], I32, tag=f"idxi{b}", name=f"idxi{b}")
                nc.vector.tensor_copy(idx_i[:], idxs[:])
                n16 = wpool.tile([128, 32], I32, tag=f"n16{b}", name=f"n16{b}")
                nc.vector.tensor_scalar(
                    n16[:], idx_i[:], 4, None,
                    op0=mybir.AluOpType.logical_shift_right)
                rb = wpool.tile([128, 32], I32, tag=f"rb{b}", name=f"rb{b}")
                nc.vector.tensor_scalar(
                    rb[:], n16[:], 1008, None, op0=mybir.AluOpType.mult)
                nc.vector.tensor_add(rb[:], rb[:], idx_i[:])
                krows = wpool.tile([128, 128], I32, tag=f"krows{b}", name=f"krows{b}")
                nc.vector.tensor_add(
                    krows[:].rearrange("p (j h) -> p j h", h=4),
                    iota[:].rearrange("p (j h) -> p j h", h=4),
                    rb[:].to_broadcast([128, 32, 4]))
                idx16 = wpool.tile([128, 128], I16, tag=f"k16{b}", name=f"k16{b}")
                nc.vector.tensor_copy(idx16[:], krows[:])

                # Gather the 2048 patch rows (192 f32 each) of this batch.
                gath = gpool.tile([128, 16 * 192], F32, tag=f"g{b}", name=f"g{b}")
                nc.gpsimd.dma_gather(
                    out_ap=gath[:].rearrange("p (m c) -> p m c", c=192),
                    in_ap=img_d[b * 16384:(b + 1) * 16384, :],
                    idxs_ap=idx16[:],
                    num_idxs=2048,
                    num_idxs_reg=2048,
                    elem_size=192,
                    single_packet=False,
                )
                # Store: gathered row g=j*64+rr sits at [64*(j%2)+rr, j//2].
                sel_v = sel_d[:].rearrange(
                    "(bb jh jl) (r c) -> bb jl r jh c", bb=B_LOCAL, jh=16, jl=2, c=192)
                for jl in range(2):
                    nc.sync.dma_start(
                        sel_v[b, jl],
                        gath[jl * 64:(jl + 1) * 64, :].rearrange(
                            "p (m c) -> p m c", c=192),
                    )

    nc.compile()
    return nc


def _get_module():
    if "nc" not in _CACHE:
        _CACHE["nc"] = _build_module()
    return _CACHE["nc"]


LAST_RESULTS = None  # BassKernelResults of the most recent kernel() call


def kernel(heatmap, image):
    global LAST_RESULTS
    heatmap = np.ascontiguousarray(np.asarray(heatmap), dtype=np.float32)
    image = np.ascontiguousarray(np.asarray(image), dtype=np.float32)
    B = heatmap.shape[0]
    assert B == B_LOCAL * N_CORES

    nc = _get_module()
    in_maps = []
    for c in range(N_CORES):
        hm = heatmap[c * B_LOCAL:(c + 1) * B_LOCAL].reshape(B_LOCAL * 1024, 1024)
        im = image[c * B_LOCAL:(c + 1) * B_LOCAL].reshape(B_LOCAL * 16384, 192)
        in_maps.append({"hm": hm, "img": im})

    trace = os.environ.get("KERNEL_PROFILE", "") == "1"
    try:
        res = run_bass_kernel_spmd(
            nc, in_maps, core_ids=list(range(N_CORES)), trace=trace)
    except ModuleNotFoundError:
        # NTFF profiling hook unavailable in this environment
        res = run_bass_kernel_spmd(
            nc, in_maps, core_ids=list(range(N_CORES)), trace=False)
    LAST_RESULTS = res
    out = np.concatenate(
        [res.results[c]["sel"].reshape(B_LOCAL * K, P, P, 3) for c in range(N_CORES)],
        axis=0)
    return out
